# revision 1
# baseline (speedup 1.0000x reference)
"""CapsuleNet forward kernel for 8 Trainium2 NeuronCores.

Data-parallel over batch (64 images / core); the routing b_ij batch-mean
uses an AllReduce per iteration.  u_hat is never materialized: s_j and the
agreement mean are computed directly against W from the 9216-dim flattened
capsule vector u.

Per-core pipeline:
  conv1  : one K=81 matmul per output tile (im2col built by a single
           strided DMA from DRAM, 2240B segments, 8 garbage cols/row that
           are cropped during the ReLU copy; conv1 bias folded into the
           ReLU's bias operand)
  conv2  : 324 accumulating K=128 matmuls (81 taps x 2 ci chunks) per co
           chunk over the full local batch (5 image-aligned PSUM banks)
  capsule: scatter-transpose conv2 output to u2T[b, f] (f = co*36+s),
           squash over 8-elem groups, PE-transpose to u2R[f, b]
  routing: s_j^T = (c-scaled W)^T @ u2, 72 K-tile accumulation;
           agreement mean m = sum_{o,i} W .* (v2^T @ u2) via rank-64
           matmul + DVE mult/group-reduce + selector matmuls;
           AllReduce(m) -> b_ij update -> softmax.
"""

import numpy as np
import ml_dtypes

import concourse.bacc as bacc
import concourse.bass as bass
import concourse.mybir as mybir
import concourse.tile as tile
from concourse.bass_utils import run_bass_kernel_spmd

F32 = mybir.dt.float32
BF16 = mybir.dt.bfloat16
MUL = mybir.AluOpType.mult
ADD = mybir.AluOpType.add
MAX = mybir.AluOpType.max
AXX = mybir.AxisListType.X
ACT = mybir.ActivationFunctionType

NCORES = 8
B = 512
BL = B // NCORES        # 64 images per core
SB = 16                 # conv1 im2col sub-batch
NSB = BL // SB
J = 560                 # 20 rows x 28 cols (8 garbage cols/row)
JC = 400                # compact 20x20 conv1 output per image
R, C, O, I = 1152, 10, 16, 8
F = R * I               # 9216
CO = C * O              # 160
KT = F // 128           # 72
S2 = 36                 # 6x6 conv2 positions per image
N2 = BL * S2
BCH = [(0, 14), (14, 14), (28, 14), (42, 14), (56, 8)]
NIT = 3


def _sub(ap, off, dims):
    """Arbitrary strided view (offset in elements, dims=[[step,count],..])."""
    return bass.AP(ap.tensor, ap.offset + off, [list(d) for d in dims])


def _pp(ap):
    """Partition pitch (elements per partition row) of an SBUF AP."""
    return ap.ap[0][0]


def build_nc(for_sim=False, reps=1):
    nc = bacc.Bacc("TRN2", target_bir_lowering=False, debug=False,
                   num_devices=1 if for_sim else NCORES)
    nc._for_sim = for_sim

    xin = nc.dram_tensor("xin", [BL * 784 + 8], BF16, kind="ExternalInput").ap()
    w1t = nc.dram_tensor("w1t", [81, 256], BF16, kind="ExternalInput").ap()
    b1 = nc.dram_tensor("b1", [128, 2], F32, kind="ExternalInput").ap()
    w2s = nc.dram_tensor("w2s", [162, 128, 256], BF16, kind="ExternalInput").ap()
    b2 = nc.dram_tensor("b2", [128, 2], F32, kind="ExternalInput").ap()
    wlb = nc.dram_tensor("wlb", [F, CO], BF16, kind="ExternalInput").ap()
    wtf = nc.dram_tensor("wtf", [CO, F], F32, kind="ExternalInput").ap()
    sel8 = nc.dram_tensor("sel8", [128, 8], F32, kind="ExternalInput").ap()
    sel2 = nc.dram_tensor("sel2", [32, 2], F32, kind="ExternalInput").ap()
    eyeb = nc.dram_tensor("eyeb", [64, 64], BF16, kind="ExternalInput").ap()
    eyef = nc.dram_tensor("eyef", [16, 16], F32, kind="ExternalInput").ap()
    out = nc.dram_tensor("out", [BL, CO], F32, kind="ExternalOutput").ap()

    selr = nc.dram_tensor("selr", [8, 128, 128], BF16, kind="ExternalInput").ap()
    cc_in = nc.dram_tensor("cc_in", [C, R], F32)
    cc_out = nc.dram_tensor("cc_out", [C, R], F32,
                            addr_space="Local" if for_sim else "Shared")
    vd = nc.dram_tensor("vd", [2, 128, N2], F32)       # conv2 out bounce

    with tile.TileContext(nc, num_cores=NCORES) as tc:
        for _rep in range(reps):
            _body(tc, nc, xin, w1t, b1, w2s, b2, wlb, wtf, sel8, sel2,
                  eyeb, eyef, selr, out, cc_in, cc_out, vd)
    nc.compile()
    return nc


def _body(tc, nc, xin, w1t, b1, w2s, b2, wlb, wtf, sel8, sel2, eyeb, eyef,
          selr, out, cc_in, cc_out, vd):
    with tc.tile_pool(name="const", bufs=1) as pc, \
         tc.tile_pool(name="upers", bufs=1) as pU:

        w1t_sb = pc.tile([81, 256], BF16, tag="w1t")
        nc.sync.dma_start(w1t_sb[:], w1t)
        b1_sb = pc.tile([128, 2], F32, tag="b1")
        nc.sync.dma_start(b1_sb[:], b1)
        b2_sb = pc.tile([128, 2], F32, tag="b2")
        nc.sync.dma_start(b2_sb[:], b2)
        sel8_sb = pc.tile([128, 8], F32, tag="sel8")
        nc.sync.dma_start(sel8_sb[:], sel8)
        sel2_sb = pc.tile([32, 2], F32, tag="sel2")
        nc.sync.dma_start(sel2_sb[:], sel2)
        eyeb_sb = pc.tile([64, 64], BF16, tag="eyeb")
        nc.sync.dma_start(eyeb_sb[:], eyeb)
        eyef_sb = pc.tile([16, 16], F32, tag="eyef")
        nc.sync.dma_start(eyef_sb[:], eyef)
        selr_sb = pc.tile([128, 8 * 128], BF16, tag="selr")
        nc.sync.dma_start(
            _sub(selr_sb[:], 0, [[_pp(selr_sb[:]), 128], [128, 8], [1, 128]]),
            _sub(selr, 0, [[128, 128], [128 * 128, 8], [1, 128]]))

        u2Tb = pU.tile([BL, F], BF16, tag="u2Tb")       # squashed u, b-major
        u2R = pU.tile([128, KT * BL], BF16, tag="u2R")  # squashed u, f-major

        # ============ Phase A: conv1 + conv2 + capsule formation ===========
        with tc.tile_pool(name="uT", bufs=1) as pT:
            u2T = pT.tile([BL, F], F32, tag="u2T")      # raw capsules, b-major

            with tc.tile_pool(name="pA", bufs=1) as pA, \
                 tc.tile_pool(name="pH", bufs=1) as pH, \
                 tc.tile_pool(name="pW2", bufs=8) as pW2, \
                 tc.tile_pool(name="pV", bufs=1) as pV, \
                 tc.tile_pool(name="ps1", bufs=2, space="PSUM") as ps1, \
                 tc.tile_pool(name="ps2", bufs=1, space="PSUM") as ps2:

                h1 = [pH.tile([128, BL * JC], BF16, tag=f"h1_{kc}",
                              name=f"h1_{kc}") for kc in range(2)]

                flip = 0
                for sb in range(NSB):
                    A = pA.tile([81, SB * J], BF16, tag="A")
                    pa = _pp(A[:])
                    for kh in range(9):
                        src = _sub(xin, sb * SB * 784 + 28 * kh,
                                   [[1, 9], [784, SB], [1, J]])
                        dst = _sub(A[:], 9 * kh * pa,
                                   [[pa, 9], [J, SB], [1, J]])
                        nc.sync.dma_start(dst, src)

                    for mc in range(2):
                        lhsT = w1t_sb[:, mc * 128:(mc + 1) * 128]
                        for bi in range(SB):
                            for hf in range(2):
                                ps = ps1.tile([128, 280], F32, tag="c1ps")
                                rhs = A[:, bi * J + hf * 280: bi * J + hf * 280 + 280]
                                nc.tensor.matmul(ps[:], lhsT, rhs,
                                                 start=True, stop=True)
                                doff = (sb * SB + bi) * JC + hf * 200
                                dstc = _sub(h1[mc][:], doff,
                                            [[_pp(h1[mc][:]), 128], [20, 10], [1, 20]])
                                srcc = _sub(ps[:], 0,
                                            [[_pp(ps[:]), 128], [28, 10], [1, 20]])
                                bb = b1_sb[:, mc:mc + 1]
                                if flip % 2 == 0:
                                    nc.vector.tensor_scalar(dstc, srcc, bb, 0.0,
                                                            op0=ADD, op1=MAX)
                                else:
                                    nc.scalar.activation(dstc, srcc, ACT.Relu,
                                                         bias=bb)
                                flip += 1

                # conv2
                for mc in range(2):
                    pss = [ps2.tile([128, nb * S2], F32, tag=f"c2ps{i}",
                                    name=f"c2ps{i}_{mc}")
                           for i, (_, nb) in enumerate(BCH)]
                    for kc in range(2):
                        for khw in range(81):
                            kh2, kw2 = khw // 9, khw % 9
                            wch = pW2.tile([128, 256], BF16, tag="wch")
                            nc.sync.dma_start(wch[:], w2s[khw * 2 + kc])
                            lhsT = wch[:, mc * 128:(mc + 1) * 128]
                            for ic, (b0, nb) in enumerate(BCH):
                                rhs = _sub(h1[kc][:], b0 * JC + 20 * kh2 + kw2,
                                           [[_pp(h1[kc][:]), 128],
                                            [JC, nb], [40, 6], [2, 6]])
                                nc.tensor.matmul(
                                    pss[ic][:], lhsT, rhs,
                                    start=(kc == 0 and khw == 0),
                                    stop=(kc == 1 and khw == 80))
                    v = pV.tile([128, N2], F32, tag="v")
                    for ic, (b0, nb) in enumerate(BCH):
                        nc.vector.tensor_scalar(v[:, b0 * S2:(b0 + nb) * S2],
                                                pss[ic][:], b2_sb[:, mc:mc + 1],
                                                None, op0=ADD)
                    # bounce via DRAM: SBUF-side DMA APs need the partition
                    # dim outermost, so the (co,b)->(b,co) transpose is done
                    # on the DRAM side
                    nc.sync.dma_start(vd.ap()[mc], v[:])
                    usrc = _sub(vd.ap(), mc * 128 * N2,
                                [[S2, BL], [N2, 128], [1, S2]])
                    udst = _sub(u2T[:], mc * 128 * S2,
                                [[_pp(u2T[:]), BL], [S2, 128], [1, S2]])
                    nc.sync.dma_start(udst, usrc)

            # ============ squash u (capsule groups of 8) ===================
            with tc.tile_pool(name="squ", bufs=1) as pq:
                sqr = pq.tile([BL, F], F32, tag="sqr")
                nc.vector.tensor_mul(sqr[:], u2T[:], u2T[:])
                sq = pq.tile([BL, R], F32, tag="sq")
                nc.vector.tensor_reduce(sq[:],
                                        sqr[:].rearrange("p (r i) -> p r i", i=I),
                                        axis=AXX, op=ADD)
                srt = pq.tile([BL, R], F32, tag="srt")
                nc.scalar.sqrt(srt[:], sq[:])
                d1 = pq.tile([BL, R], F32, tag="d1")
                nc.vector.tensor_scalar(d1[:], sq[:], 1.0, None, op0=ADD)
                d2 = pq.tile([BL, R], F32, tag="d2")
                nc.vector.tensor_mul(d2[:], d1[:], srt[:])
                rc = pq.tile([BL, R], F32, tag="rc")
                nc.vector.reciprocal(rc[:], d2[:])
                g = pq.tile([BL, R], F32, tag="g")
                nc.vector.tensor_mul(g[:], sq[:], rc[:])
                # u2Tb = u2T * g, one strided pass per capsule element
                ppu = _pp(u2T[:])
                ppb = _pp(u2Tb[:])
                for i in range(I):
                    nc.vector.tensor_tensor(
                        _sub(u2Tb[:], i, [[ppb, BL], [I, R]]),
                        _sub(u2T[:], i, [[ppu, BL], [I, R]]),
                        g[:], op=MUL)

        # ============ u2R = transpose(u2Tb) ================================
        with tc.tile_pool(name="ptr", bufs=2, space="PSUM") as ptr:
            for t in range(KT):
                pst = ptr.tile([128, BL], BF16, tag="tr")
                nc.tensor.transpose(pst[:], u2Tb[:, t * 128:(t + 1) * 128],
                                    eyeb_sb[:])
                nc.vector.tensor_copy(u2R[:, t * BL:(t + 1) * BL], pst[:])

        # ============ routing ==============================================
        with tc.tile_pool(name="pB", bufs=1) as pB, \
             tc.tile_pool(name="pBs", bufs=2) as pBs, \
             tc.tile_pool(name="psq2", bufs=1) as pq, \
             tc.tile_pool(name="psB", bufs=2, space="PSUM") as psB, \
             tc.tile_pool(name="psS", bufs=1, space="PSUM") as psS:

            wsb = pB.tile([128, KT * CO], BF16, tag="wsb")
            wsrc = _sub(wlb, 0, [[CO, 128], [128 * CO, KT], [1, CO]])
            wdst = _sub(wsb[:], 0, [[_pp(wsb[:]), 128], [CO, KT], [1, CO]])
            nc.sync.dma_start(wdst, wsrc)
            wt0 = pB.tile([128, F], F32, tag="wt0")
            nc.sync.dma_start(wt0[:], wtf[0:128])
            wt1 = pB.tile([32, F], F32, tag="wt1")
            nc.sync.dma_start(wt1[:], wtf[128:160])
            wp = pB.tile([128, KT * CO], BF16, tag="wp")
            cE = pB.tile([128, KT * C], BF16, tag="cE")
            cTr = pB.tile([128, 9 * C], BF16, tag="cTr")
            mAll = pB.tile([8, R], F32, tag="mAll")
            mAll2 = pB.tile([2, R], F32, tag="mAll2")
            bijA = pB.tile([C, R], F32, tag="bijA")
            bijB = pB.tile([C, R], F32, tag="bijB")
            csm = pB.tile([C, R], F32, tag="csm")
            v2T = pB.tile([BL, CO], F32, tag="v2T")
            v2Tb = pB.tile([BL, CO], BF16, tag="v2Tb")
            msum = pB.tile([C, R], F32, tag="msum")

            lam = 1.0 / R
            for it in range(NIT):
                if it > 0:
                    # cTr[r%128, q*10+c] = csm[c, r]  (PE transpose, 9 blocks)
                    for q in range(9):
                        pst = psB.tile([128, C], F32, tag="ctr", name="ctr", bufs=1)
                        nc.tensor.transpose(pst[:],
                                            csm[:, q * 128:(q + 1) * 128],
                                            eyef_sb[0:C, 0:C])
                        nc.vector.tensor_copy(cTr[:, q * C:(q + 1) * C], pst[:])
                    # cE[8r''+i, (8t2+t1)*10+c] = cTr[16*t1+r'', t2*10+c]
                    # via selector matmuls: SEL_t1[k, p] = (k == 16*t1 + p//8)
                    for t1 in range(8):
                        pse = psB.tile([128, 9 * C], F32, tag="cexp", name="cexp", bufs=1)
                        nc.tensor.matmul(pse[:],
                                         selr_sb[:, t1 * 128:(t1 + 1) * 128],
                                         cTr[:], start=True, stop=True)
                        nc.vector.tensor_copy(
                            _sub(cE[:], t1 * C,
                                 [[_pp(cE[:]), 128], [8 * C, 9], [1, C]]),
                            pse[:])
                    # wp = wsb * cE, one strided pass per o
                    ppw = _pp(wp[:])
                    pps = _pp(wsb[:])
                    for o in range(O):
                        nc.vector.tensor_tensor(
                            _sub(wp[:], o, [[ppw, 128], [CO, KT], [O, C]]),
                            _sub(wsb[:], o, [[pps, 128], [CO, KT], [O, C]]),
                            cE[:].rearrange("p (t c) -> p t c", c=C), op=MUL)

                # s_j^T [b, co] over 72 accumulating K-tiles
                wcur = wsb if it == 0 else wp
                ssum = psS.tile([BL, CO], F32, tag="ssum")
                for t in range(KT):
                    nc.tensor.matmul(ssum[:], u2R[:, t * BL:(t + 1) * BL],
                                     wcur[:, t * CO:(t + 1) * CO],
                                     start=(t == 0), stop=(t == KT - 1))

                # v2 = squash(s) over o-groups of 16 (iter0 folds the 1/R scale)
                ssb = pq.tile([BL, CO], F32, tag="ssb")
                nc.vector.tensor_copy(ssb[:], ssum[:])
                svr = pq.tile([BL, CO], F32, tag="svr")
                nc.vector.tensor_mul(svr[:], ssb[:], ssb[:])
                sqv = pq.tile([BL, C], F32, tag="sqv")
                nc.vector.tensor_reduce(sqv[:],
                                        svr[:].rearrange("p (c o) -> p c o", o=O),
                                        axis=AXX, op=ADD)
                if it == 0:
                    nc.vector.tensor_scalar(sqv[:], sqv[:], lam * lam, None, op0=MUL)
                srtv = pq.tile([BL, C], F32, tag="srtv")
                nc.scalar.sqrt(srtv[:], sqv[:])
                dv1 = pq.tile([BL, C], F32, tag="dv1")
                nc.vector.tensor_scalar(dv1[:], sqv[:], 1.0, None, op0=ADD)
                dv2 = pq.tile([BL, C], F32, tag="dv2")
                nc.vector.tensor_mul(dv2[:], dv1[:], srtv[:])
                rcv = pq.tile([BL, C], F32, tag="rcv")
                nc.vector.reciprocal(rcv[:], dv2[:])
                gv = pq.tile([BL, C], F32, tag="gv")
                nc.vector.tensor_mul(gv[:], sqv[:], rcv[:])
                if it == 0:
                    nc.vector.tensor_scalar(gv[:], gv[:], lam, None, op0=MUL)
                ppv = _pp(v2T[:])
                pps2 = _pp(ssb[:])
                for o in range(O):
                    nc.vector.tensor_tensor(
                        _sub(v2T[:], o, [[ppv, BL], [O, C]]),
                        _sub(ssb[:], o, [[pps2, BL], [O, C]]),
                        gv[:], op=MUL)

                if it == NIT - 1:
                    nc.sync.dma_start(out, v2T[:])
                    break

                nc.vector.tensor_copy(v2Tb[:], v2T[:])
                # m[c, r] = sum_{o,i} Wt[(c,o),(r,i)] * (v2^T @ u2)[(c,o),(r,i)]
                for mc2 in range(2):
                    npart = 128 if mc2 == 0 else 32
                    ncls = 8 if mc2 == 0 else 2
                    lhs = v2Tb[:, mc2 * 128: mc2 * 128 + npart]
                    selt = (sel8_sb if mc2 == 0 else sel2_sb)[0:npart, 0:ncls]
                    wtt = wt0 if mc2 == 0 else wt1
                    for nch in range(18):
                        f0 = nch * 512
                        tps = psB.tile([128, 512], F32, tag="tprime")
                        nc.tensor.matmul(tps[0:npart, :], lhs,
                                         u2Tb[:, f0:f0 + 512],
                                         start=True, stop=True)
                        pm = pBs.tile([128, 512], F32, tag="pm")
                        nc.vector.tensor_tensor(pm[0:npart, :],
                                                wtt[0:npart, f0:f0 + 512],
                                                tps[0:npart, :], op=MUL)
                        pr = pBs.tile([128, 64], F32, tag="pr")
                        nc.vector.tensor_reduce(
                            pr[0:npart, :],
                            pm[0:npart, :].rearrange("p (r i) -> p r i", i=I),
                            axis=AXX, op=ADD)
                        mo = psB.tile([16, 64], F32, tag="mo", bufs=2)
                        nc.tensor.matmul(mo[0:ncls, :], selt, pr[0:npart, :],
                                         start=True, stop=True)
                        mtgt = mAll if mc2 == 0 else mAll2
                        nc.vector.tensor_copy(
                            mtgt[0:ncls, f0 // I: f0 // I + 64],
                            mo[0:ncls, :])

                nc.sync.dma_start(cc_in.ap()[0:8], mAll[:])
                nc.sync.dma_start(cc_in.ap()[8:10], mAll2[:])
                if getattr(nc, "_for_sim", False):
                    nc.sync.dma_start(cc_out.ap(), cc_in.ap())
                else:
                    nc.gpsimd.collective_compute(
                        "AllReduce", ADD,
                        replica_groups=[list(range(NCORES))],
                        ins=[cc_in.ap()], outs=[cc_out.ap()])
                nc.sync.dma_start(msum[:], cc_out.ap())
                bij = bijA if it == 0 else bijB
                if it == 0:
                    nc.vector.tensor_scalar(bij[:], msum[:], 1.0 / B, None, op0=MUL)
                else:
                    nc.vector.tensor_scalar(bij[:], msum[:], 1.0 / B, None, op0=MUL)
                    nc.vector.tensor_add(bij[:], bij[:], bijA[:])
                # softmax over routes (free dim)
                rmax = pq.tile([C, 1], F32, tag="rmax")
                nc.vector.tensor_reduce(rmax[:], bij[:], axis=AXX, op=MAX)
                nrm = pq.tile([C, 1], F32, tag="nrm")
                nc.vector.tensor_scalar(nrm[:], rmax[:], -1.0, None, op0=MUL)
                nc.scalar.activation(csm[:], bij[:], ACT.Exp, bias=nrm[:])
                rsm = pq.tile([C, 1], F32, tag="rsm")
                nc.vector.tensor_reduce(rsm[:], csm[:], axis=AXX, op=ADD)
                rrc = pq.tile([C, 1], F32, tag="rrc")
                nc.vector.reciprocal(rrc[:], rsm[:])
                nc.vector.tensor_scalar(csm[:], csm[:], rrc[:], None, op0=MUL)


# ------------------------- host side ---------------------------------------
_CACHE = {}


def kernel(x, conv1_w, conv1_b, conv2_w, conv2_b, W):
    if "nc" not in _CACHE:
        _CACHE["nc"] = build_nc()
    nc = _CACHE["nc"]

    bf = ml_dtypes.bfloat16
    xf = np.ascontiguousarray(np.asarray(x, np.float32).reshape(B, 784))
    w1 = np.ascontiguousarray(
        np.asarray(conv1_w, np.float32).reshape(256, 81).T).astype(bf)
    b1v = np.asarray(conv1_b, np.float32).reshape(2, 128).T.copy()
    w2 = np.asarray(conv2_w, np.float32).reshape(256, 256, 81)
    w2 = np.ascontiguousarray(w2.transpose(2, 1, 0)).reshape(162, 128, 256).astype(bf)
    b2v = np.asarray(conv2_b, np.float32).reshape(2, 128).T.copy()
    Wf = np.asarray(W, np.float32)
    wl = np.ascontiguousarray(Wf.transpose(0, 3, 1, 2)).reshape(F, CO).astype(bf)
    wt = np.ascontiguousarray(Wf.transpose(1, 2, 0, 3)).reshape(CO, F).astype(np.float32)
    s8 = np.zeros((128, 8), np.float32)
    s8[np.arange(128), np.arange(128) // 16] = 1.0
    s2m = np.zeros((32, 2), np.float32)
    s2m[np.arange(32), np.arange(32) // 16] = 1.0
    srn = np.zeros((8, 128, 128), np.float32)
    for t1 in range(8):
        srn[t1, 16 * t1 + np.arange(128) // 8, np.arange(128)] = 1.0

    shared = {
        "w1t": w1, "b1": b1v, "w2s": w2, "b2": b2v, "wlb": wl, "wtf": wt,
        "sel8": s8, "sel2": s2m, "selr": srn.astype(bf),
        "eyeb": np.eye(64).astype(bf), "eyef": np.eye(16, dtype=np.float32),
    }
    in_maps = []
    for c in range(NCORES):
        xs = np.zeros(BL * 784 + 8, bf)
        xs[:BL * 784] = xf[c * BL:(c + 1) * BL].reshape(-1).astype(bf)
        in_maps.append({"xin": xs, **shared})
    res = run_bass_kernel_spmd(nc, in_maps, list(range(NCORES)), trace=False)
    outs = [res.results[c]["out"] for c in range(NCORES)]
    return np.concatenate(outs, axis=0).reshape(B, C, O).astype(np.float32)



# revision 30
# speedup vs baseline: 2.1314x; 2.1314x over previous
"""CapsuleNet forward kernel for 8 Trainium2 NeuronCores.

Data-parallel over batch (64 images / core); the routing b_ij batch-mean
uses an AllReduce per iteration.  u_hat is never materialized: s_j and the
agreement mean are computed directly against W from the 9216-dim flattened
capsule vector u.

Per-core pipeline:
  conv1  : K=81 matmuls with garbage-cropped 200-col rhs (im2col built by
           strided DMA from DRAM, 2240B segments); 8 matmul slots per
           4-bank PSUM tile; one fused ReLU+bias copy per 4 images,
           alternating DVE/Act engines
  conv2  : 324 accumulating K=128 matmuls (81 taps x 2 ci chunks) per co
           chunk over the full local batch (5 image-aligned PSUM banks);
           bias-add drains to bf16 and scatter-writes the capsule layout
           to DRAM (72B segments)
  capsule: u2T[p = b + 64*mc, co*36+s] bf16 [128, 4608]; squash over
           8-elem groups (f32 norms, bf16 scale); u2R = xbar DMA-transpose
           (f-major, cols (h, j, b))
  routing: s_j^T = (c-scaled W)^T @ u2, 72 K-tile accumulation;
           agreement mean m = sum_{o,i} W .* (v2^T @ u2) via rank-64
           matmul + DVE mult/group-reduce + selector matmuls;
           AllReduce(m) -> b_ij update -> softmax.
"""

import numpy as np
import ml_dtypes

import concourse.bacc as bacc
import concourse.bass as bass
import concourse.mybir as mybir
import concourse.tile as tile
from concourse.bass_utils import run_bass_kernel_spmd

F32 = mybir.dt.float32
BF16 = mybir.dt.bfloat16
MUL = mybir.AluOpType.mult
ADD = mybir.AluOpType.add
MAX = mybir.AluOpType.max
AXX = mybir.AxisListType.X
ACT = mybir.ActivationFunctionType

NCORES = 8
B = 512
BL = B // NCORES        # 64 images per core
SB = 16                 # conv1 im2col sub-batch
NSB = BL // SB
J = 560                 # 20 rows x 28 cols (8 garbage cols/row)
JC = 400                # compact 20x20 conv1 output per image
R, C, O, I = 1152, 10, 16, 8
F = R * I               # 9216
FH = F // 2             # 4608 per fold half
CO = C * O              # 160
KT = F // 128           # 72
S2 = 36                 # 6x6 conv2 positions per image
N2 = BL * S2
BCH = [(0, 14), (14, 14), (28, 14), (42, 14), (56, 8)]
NIT = 3


def _sub(ap, off, dims):
    """Arbitrary strided view (offset in elements, dims=[[step,count],..])."""
    return bass.AP(ap.tensor, ap.offset + off, [list(d) for d in dims])


def _pp(ap):
    """Partition pitch (elements per partition row) of an SBUF AP."""
    return ap.ap[0][0]


def build_nc(for_sim=False, reps=1):
    nc = bacc.Bacc("TRN2", target_bir_lowering=False, debug=False,
                   num_devices=1 if for_sim else NCORES)
    nc._for_sim = for_sim

    xin = nc.dram_tensor("xin", [BL * 784 + 8], BF16, kind="ExternalInput").ap()
    w1t = nc.dram_tensor("w1t", [81, 256], BF16, kind="ExternalInput").ap()
    b1 = nc.dram_tensor("b1", [128, 2], F32, kind="ExternalInput").ap()
    w2s = nc.dram_tensor("w2s", [162, 128, 256], BF16, kind="ExternalInput").ap()
    b2 = nc.dram_tensor("b2", [128, 2], F32, kind="ExternalInput").ap()
    wlb = nc.dram_tensor("wlb", [128, KT * CO], BF16, kind="ExternalInput").ap()
    sel8x = nc.dram_tensor("sel8x", [128, 16], BF16, kind="ExternalInput").ap()
    selc = nc.dram_tensor("selc", [16, 128], BF16, kind="ExternalInput").ap()
    ones16 = nc.dram_tensor("ones16", [16, 1], F32, kind="ExternalInput").ap()
    ones1 = nc.dram_tensor("ones1", [1, 16], F32, kind="ExternalInput").ap()
    eyeb2 = nc.dram_tensor("eyeb2", [64, 128], BF16, kind="ExternalInput").ap()
    out = nc.dram_tensor("out", [BL, CO], F32, kind="ExternalOutput").ap()

    cc_in = nc.dram_tensor("cc_in", [16, KT * C], F32)
    cc_out = nc.dram_tensor("cc_out", [16, KT * C], F32,
                            addr_space="Local" if for_sim else "Shared")
    vd = nc.dram_tensor("vd", [128, FH], BF16)         # conv2 out bounce

    with tile.TileContext(nc, num_cores=NCORES) as tc:
        for _rep in range(reps):
            _body(tc, nc, xin, w1t, b1, w2s, b2, wlb, sel8x, selc,
                  ones16, ones1, eyeb2, out, cc_in, cc_out, vd)
    nc.compile()
    return nc


def _body(tc, nc, xin, w1t, b1, w2s, b2, wlb, sel8x, selc,
          ones16, ones1, eyeb2, out, cc_in, cc_out, vd):
    with tc.tile_pool(name="const", bufs=1) as pc, \
         tc.tile_pool(name="upers", bufs=1) as pU:

        w1t_sb = pc.tile([81, 256], BF16, tag="w1t")
        nc.scalar.dma_start(w1t_sb[:], w1t)
        b1_sb = pc.tile([128, 2], F32, tag="b1")
        nc.scalar.dma_start(b1_sb[:], b1)
        b2_sb = pc.tile([128, 2], F32, tag="b2")
        nc.scalar.dma_start(b2_sb[:], b2)
        sel8x_sb = pc.tile([128, 16], BF16, tag="sel8x")
        nc.scalar.dma_start(sel8x_sb[:], sel8x)
        selc_sb = pc.tile([16, 128], BF16, tag="selc")
        nc.scalar.dma_start(selc_sb[:], selc)
        ones16_sb = pc.tile([16, 1], F32, tag="ones16")
        nc.scalar.dma_start(ones16_sb[:], ones16)
        ones1_sb = pc.tile([1, 16], F32, tag="ones1")
        nc.scalar.dma_start(ones1_sb[:], ones1)
        eyeb2_sb = pc.tile([64, 128], BF16, tag="eyeb2")
        nc.scalar.dma_start(eyeb2_sb[:], eyeb2)
        u2T = pU.tile([128, FH], BF16, tag="u2T")       # folded capsules
        u2R = pU.tile([128, FH], BF16, tag="u2R")       # f-major (h, j, b)
        wsb = pU.tile([128, KT * CO], BF16, tag="wsb")

        # ============ Phase A: conv1 + conv2 + capsule formation ===========
        with tc.tile_pool(name="pH", bufs=1) as pH:
            h1 = [pH.tile([128, BL * JC], BF16, tag=f"h1_{kc}",
                          name=f"h1_{kc}") for kc in range(2)]

            # conv1: 8 matmul slots (4 images x 2 half-rows) per 4-bank
            # PSUM tile; one fused bias+ReLU copy per tile
            with tc.tile_pool(name="pA", bufs=2) as pA, \
                 tc.tile_pool(name="ps1", bufs=4, space="PSUM") as ps1:
                flip = 0
                for half in range(NSB):
                    b0h = half * SB
                    A = pA.tile([81, SB * J], BF16, tag="A")
                    pa = _pp(A[:])
                    # im2col: one DMA per kernel row
                    for kh in range(9):
                        nc.sync.dma_start(
                            _sub(A[:], 9 * kh * pa, [[pa, 9], [J, SB], [1, J]]),
                            _sub(xin, b0h * 784 + 28 * kh,
                                 [[1, 9], [784, SB], [1, J]]))

                    for bg in range(SB // 2):
                        for mc in range(2):
                            lhsT = w1t_sb[:, mc * 128:(mc + 1) * 128]
                            ps = ps1.tile([128, 1024], F32, tag="c1ps")
                            for sl in range(4):      # slot = (img, hf)
                                bi = bg * 2 + sl // 2
                                hf = sl % 2
                                rhs = _sub(A[:], bi * J + hf * 280,
                                           [[pa, 81], [28, 10], [1, 20]])
                                nc.tensor.matmul(
                                    ps[:, sl * 256: sl * 256 + 200],
                                    lhsT, rhs, start=True, stop=True)
                            doff = (b0h + bg * 2) * JC
                            dstc = _sub(h1[mc][:], doff,
                                        [[_pp(h1[mc][:]), 128], [200, 4],
                                         [1, 200]])
                            srcc = _sub(ps[:], 0,
                                        [[_pp(ps[:]), 128], [256, 4], [1, 200]])
                            bb = b1_sb[:, mc:mc + 1]
                            if flip % 2 == 0:
                                nc.vector.tensor_scalar(dstc, srcc, bb, 0.0,
                                                        op0=ADD, op1=MAX)
                            else:
                                nc.scalar.activation(dstc, srcc, ACT.Relu,
                                                     bias=bb)
                            flip += 1

            # routing weights: contiguous loads on the Act DMA queue,
            # floored past conv1's im2col so they don't steal DMA bandwidth
            with tc.tile_wait_until(0.012):
                nc.scalar.dma_start(wsb[:], wlb)

            # conv2 (the mc=0 half's squash hides under mc=1's matmuls)
            with tc.tile_pool(name="pW2", bufs=8) as pW2, \
                 tc.tile_pool(name="pV", bufs=2) as pV, \
                 tc.tile_pool(name="squ", bufs=2) as pq0, \
                 tc.tile_pool(name="ps2", bufs=1, space="PSUM") as ps2:
                NGC = 1152 // I          # squash chunk: 1152 f, 144 groups

                def squash_half(h):
                    # squash u2T rows [64h, 64h+64) in 4 column chunks,
                    # then DMA-transpose the half into u2R
                    rows = slice(h * 64, h * 64 + 64)
                    for qc in range(4):
                        fsl = slice(qc * 1152, (qc + 1) * 1152)
                        uv = u2T[rows, fsl]
                        sqr = pq0.tile([128, 1152], F32, tag="sqr", name="sqr")[rows, :]
                        nc.vector.tensor_mul(sqr, uv, uv)
                        sq = pq0.tile([128, NGC], F32, tag="sq", name="sq")[rows, :]
                        nc.vector.tensor_reduce(
                            sq, sqr.rearrange("p (r i) -> p r i", i=I),
                            axis=AXX, op=ADD)
                        srt = pq0.tile([128, NGC], F32, tag="srt", name="srt")[rows, :]
                        nc.scalar.sqrt(srt, sq)
                        d2 = pq0.tile([128, NGC], F32, tag="d2", name="d2")[rows, :]
                        nc.vector.scalar_tensor_tensor(d2, sq, 1.0, srt,
                                                       op0=ADD, op1=MUL)
                        rc = pq0.tile([128, NGC], F32, tag="rc", name="rc")[rows, :]
                        nc.vector.reciprocal(rc, d2)
                        g = pq0.tile([128, NGC], F32, tag="g", name="g")[rows, :]
                        nc.vector.tensor_mul(g, sq, rc)
                        gx = pq0.tile([128, 1152], BF16, tag="gx", name="gx")[rows, :]
                        ppg = _pp(g)
                        ppx = _pp(gx)
                        nc.scalar.activation(
                            _sub(gx, 0, [[ppx, 64], [I, NGC], [1, I]]),
                            _sub(g, 0, [[ppg, 64], [1, NGC], [0, I]]),
                            ACT.Copy)
                        nc.vector.tensor_mul(uv, uv, gx)
                    # u2R[q, h*2304 + j*64 + b] = u2T[b + 64h, j*128 + q]
                    nc.sync.dma_start_transpose(
                        _sub(u2R[:], h * 2304,
                             [[_pp(u2R[:]), 128], [64, 36], [1, 64]]),
                        u2T[rows, :])

                for mc in range(2):
                    pss = [ps2.tile([128, nb * S2], F32, tag=f"c2ps{i}",
                                    name=f"c2ps{i}_{mc}")
                           for i, (_, nb) in enumerate(BCH)]
                    for khw in range(81):
                        kh2, kw2 = khw // 9, khw % 9
                        wch = pW2.tile([128, 512], BF16, tag="wch")
                        nc.sync.dma_start(
                            _sub(wch[:], 0,
                                 [[_pp(wch[:]), 128], [256, 2], [1, 256]]),
                            _sub(w2s, khw * 2 * 128 * 256,
                                 [[256, 128], [128 * 256, 2], [1, 256]]))
                        for kc in range(2):
                            lhsT = wch[:, kc * 256 + mc * 128:
                                        kc * 256 + mc * 128 + 128]
                            for ic, (b0, nb) in enumerate(BCH):
                                rhs = _sub(h1[kc][:], b0 * JC + 20 * kh2 + kw2,
                                           [[_pp(h1[kc][:]), 128],
                                            [JC, nb], [40, 6], [2, 6]])
                                nc.tensor.matmul(
                                    pss[ic][:], lhsT, rhs,
                                    start=(kc == 0 and khw == 0),
                                    stop=(kc == 1 and khw == 80))
                    v = pV.tile([128, N2], BF16, tag="v")
                    for ic, (b0, nb) in enumerate(BCH):
                        nc.scalar.activation(v[:, b0 * S2:(b0 + nb) * S2],
                                             pss[ic][:], ACT.Identity,
                                             bias=b2_sb[:, mc:mc + 1])
                    # scatter-write the capsule layout: vd[b+64mc, co*36+s]
                    vdst = _sub(vd.ap(), mc * 64 * FH,
                                [[36, 128], [FH, 64], [1, 36]])
                    nc.sync.dma_start(vdst, v[:])
                    # contiguous read back into the folded SBUF tile
                    nc.sync.dma_start(u2T[mc * 64:(mc + 1) * 64, :],
                                      _sub(vd.ap(), mc * 64 * FH,
                                           [[FH, 64], [1, FH]]))
                    squash_half(mc)

        if True:
            # ============ routing ==========================================
            # co-order is (o, c): co' = o*10 + c. m/b_ij/csm live in the
            # [16 r_local, 72 t * 10 c] layout; r = 16t + r_local.
            with tc.tile_pool(name="pB", bufs=1) as pB, \
                 tc.tile_pool(name="pTb", bufs=3) as pTb, \
                 tc.tile_pool(name="pPm", bufs=3) as pPm, \
                 tc.tile_pool(name="psq2", bufs=1) as pq, \
                 tc.tile_pool(name="psB", bufs=2, space="PSUM") as psB, \
                 tc.tile_pool(name="psS", bufs=1, space="PSUM") as psS:

                wp = pB.tile([128, KT * CO], BF16, tag="wp")
                cE = pB.tile([128, KT * C], BF16, tag="cE")
                bijA = pB.tile([16, KT * C], F32, tag="bijA")
                bijB = pB.tile([16, KT * C], F32, tag="bijB")
                exp16 = pB.tile([16, KT * C], F32, tag="exp16")
                csm16 = pB.tile([16, KT * C], BF16, tag="csm16")
                mAllN = pB.tile([16, KT * C], F32, tag="mAllN")
                msum = pB.tile([16, KT * C], F32, tag="msum")
                pro = pB.tile([128, KT * C], BF16, tag="pro")
                v2T = pB.tile([BL, CO], F32, tag="v2T")
                v2Tb = pB.tile([BL, CO], BF16, tag="v2Tb")
                v2rep = pB.tile([128, CO], BF16, tag="v2rep")

                lam = 1.0 / R
                for it in range(NIT):
                    if it > 0:
                        # cE[8*rl+i, (t,c)] = csm16[rl, (t,c)] via selector
                        ceps = psB.tile([128, 1024], F32, tag="ceps", bufs=1)
                        for hf2 in range(2):
                            nc.tensor.matmul(
                                ceps[:, hf2 * 512: hf2 * 512 + 360],
                                selc_sb[:], csm16[:, hf2 * 360:(hf2 + 1) * 360],
                                start=True, stop=True)
                            nc.scalar.activation(
                                cE[:, hf2 * 360:(hf2 + 1) * 360],
                                ceps[:, hf2 * 512: hf2 * 512 + 360], ACT.Copy)
                        # wp = wsb * broadcast_o(cE), single 2x pass
                        ppw = _pp(wp[:])
                        pps = _pp(wsb[:])
                        ppe = _pp(cE[:])
                        nc.vector.tensor_tensor(
                            _sub(wp[:], 0, [[ppw, 128], [CO, KT], [C, O], [1, C]]),
                            _sub(wsb[:], 0, [[pps, 128], [CO, KT], [C, O], [1, C]]),
                            _sub(cE[:], 0, [[ppe, 128], [C, KT], [0, O], [1, C]]),
                            op=MUL)

                    # s_j^T [b, (o,c)] over 72 accumulating K-tiles
                    wcur = wsb if it == 0 else wp
                    ssum = psS.tile([128, CO], F32, tag="sv")
                    for t in range(KT):
                        j, h = t % 36, t // 36
                        lhsT = _sub(u2R[:], h * 2304 + j * 64,
                                    [[_pp(u2R[:]), 128], [1, BL]])
                        nc.tensor.matmul(ssum[0:BL, :], lhsT,
                                         wcur[:, t * CO:(t + 1) * CO],
                                         start=(t == 0), stop=(t == KT - 1))

                    # v2 = squash(s) over o-groups (iter0 folds 1/R)
                    ssb = pq.tile([BL, CO], F32, tag="ssb")
                    nc.vector.tensor_copy(ssb[:], ssum[0:BL, :])
                    svr = pq.tile([BL, CO], F32, tag="svr")
                    nc.vector.tensor_mul(svr[:], ssb[:], ssb[:])
                    sqv = pq.tile([BL, C], F32, tag="sqv")
                    ppsv = _pp(svr[:])
                    nc.vector.tensor_reduce(
                        sqv[:],
                        _sub(svr[:], 0, [[ppsv, BL], [1, C], [C, O]]),
                        axis=AXX, op=ADD)
                    if it == 0:
                        nc.vector.tensor_scalar(sqv[:], sqv[:], lam * lam,
                                                None, op0=MUL)
                    srtv = pq.tile([BL, C], F32, tag="srtv")
                    nc.scalar.sqrt(srtv[:], sqv[:])
                    dv2 = pq.tile([BL, C], F32, tag="dv2")
                    nc.vector.scalar_tensor_tensor(dv2[:], sqv[:], 1.0,
                                                   srtv[:], op0=ADD, op1=MUL)
                    rcv = pq.tile([BL, C], F32, tag="rcv")
                    nc.vector.reciprocal(rcv[:], dv2[:])
                    gv = pq.tile([BL, C], F32, tag="gv")
                    nc.vector.tensor_mul(gv[:], sqv[:], rcv[:])
                    if it == 0:
                        nc.vector.tensor_scalar(gv[:], gv[:], lam, None,
                                                op0=MUL)
                    ppv = _pp(v2T[:])
                    pps2 = _pp(ssb[:])
                    ppgv = _pp(gv[:])
                    nc.vector.tensor_tensor(
                        _sub(v2T[:], 0, [[ppv, BL], [C, O], [1, C]]),
                        _sub(ssb[:], 0, [[pps2, BL], [C, O], [1, C]]),
                        _sub(gv[:], 0, [[ppgv, BL], [0, O], [1, C]]),
                        op=MUL)

                    if it == NIT - 1:
                        # out stays in (o, c) order; host transposes
                        nc.sync.dma_start(out, v2T[:])
                        break

                    # v replicated to both partition halves via PE selector
                    nc.scalar.activation(v2Tb[:], v2T[:], ACT.Copy)
                    vrp = psS.tile([128, CO], F32, tag="sv")
                    nc.tensor.matmul(vrp[:], eyeb2_sb[:], v2Tb[:],
                                     start=True, stop=True)
                    nc.scalar.activation(v2rep[:], vrp[:], ACT.Copy)

                    # agreement: T'f[f, (o,c)] = sum_b u[b,f] v[b,(o,c)],
                    # then m16[rl, (t,c)] = sum_{i,o} wsb .* T'f
                    for ggr in range(KT // 3):
                        tfp = psB.tile([128, 480], F32, tag="tfp")
                        for dt3 in range(3):
                            t = ggr * 3 + dt3
                            j, h = t % 36, t // 36
                            lhsT = u2T[h * 64:(h + 1) * 64,
                                       j * 128:(j + 1) * 128]
                            nc.tensor.matmul(tfp[:, dt3 * CO:(dt3 + 1) * CO],
                                             lhsT,
                                             v2rep[h * 64:(h + 1) * 64, :],
                                             start=True, stop=True)
                        tpb = pTb.tile([128, 480], BF16, tag="tpb")
                        nc.scalar.activation(tpb[:], tfp[:], ACT.Copy)
                        pm = pPm.tile([128, 480], BF16, tag="pm")
                        mule = nc.vector if ggr % 2 == 0 else nc.gpsimd
                        mule.tensor_tensor(
                            pm[:], tpb[:], wsb[:, ggr * 480:(ggr + 1) * 480],
                            op=MUL)
                        ppm = _pp(pm[:])
                        ppr = _pp(pro[:])
                        with nc.allow_low_precision(reason="m16 in bf16 ok"):
                            nc.vector.tensor_reduce(
                                _sub(pro[:], ggr * 30,
                                     [[ppr, 128], [10, 3], [1, 10]]),
                                _sub(pm[:], 0,
                                     [[ppm, 128], [CO, 3], [1, C], [C, O]]),
                                axis=AXX, op=ADD)
                    # i-sum via selector: m16[rl, (t,c)]
                    m16p = psB.tile([16, 1024], F32, tag="m16p", bufs=1)
                    for hf2 in range(2):
                        nc.tensor.matmul(
                            m16p[:, hf2 * 512: hf2 * 512 + 360],
                            sel8x_sb[:], pro[:, hf2 * 360:(hf2 + 1) * 360],
                            start=True, stop=True)
                    ppmp = _pp(m16p[:])
                    nc.vector.tensor_copy(
                        mAllN[:],
                        _sub(m16p[:], 0, [[ppmp, 16], [512, 2], [1, 360]]))

                    nc.sync.dma_start(cc_in.ap(), mAllN[:])
                    if getattr(nc, "_for_sim", False):
                        nc.sync.dma_start(cc_out.ap(), cc_in.ap())
                    else:
                        nc.gpsimd.collective_compute(
                            "AllReduce", ADD,
                            replica_groups=[list(range(NCORES))],
                            ins=[cc_in.ap()], outs=[cc_out.ap()])
                    nc.sync.dma_start(msum[:], cc_out.ap())
                    bij = bijA if it == 0 else bijB
                    if it == 0:
                        nc.vector.tensor_scalar(bij[:], msum[:], 1.0 / B,
                                                None, op0=MUL)
                    else:
                        nc.vector.scalar_tensor_tensor(
                            bij[:], msum[:], 1.0 / B, bijA[:],
                            op0=MUL, op1=ADD)
                    # softmax over routes r = (rl, t); no max-subtraction
                    # (|b_ij| stays O(1), far from the exp overflow range)
                    nc.scalar.activation(exp16[:], bij[:], ACT.Exp)
                    tsum = pq.tile([16, C], F32, tag="tsum")
                    ppb = _pp(exp16[:])
                    nc.vector.tensor_reduce(
                        tsum[:],
                        _sub(exp16[:], 0, [[ppb, 16], [1, C], [C, KT]]),
                        axis=AXX, op=ADD)
                    # partition-sum (16 -> 1) and broadcast back (1 -> 16)
                    dsp = psS.tile([16, 128], F32, tag="dsp")
                    nc.tensor.matmul(dsp[0:1, 0:C], ones16_sb[:], tsum[:],
                                     start=True, stop=True)
                    rcp = pq.tile([1, C], F32, tag="rcp")
                    nc.vector.reciprocal(rcp[:], dsp[0:1, 0:C])
                    nc.tensor.matmul(dsp[0:16, 64:64 + C], ones1_sb[:],
                                     rcp[:], start=True, stop=True)
                    ppd = _pp(dsp[:])
                    nc.vector.tensor_tensor(
                        csm16[:], exp16[:],
                        _sub(dsp[:], 64, [[ppd, 16], [0, KT], [1, C]]),
                        op=MUL)


# ------------------------- host side ---------------------------------------
_CACHE = {}


def make_in_maps(x, conv1_w, conv1_b, conv2_w, conv2_b, W):
    bf = ml_dtypes.bfloat16
    xf = np.ascontiguousarray(np.asarray(x, np.float32).reshape(B, 784))
    w1 = np.ascontiguousarray(
        np.asarray(conv1_w, np.float32).reshape(256, 81).T).astype(bf)
    b1v = np.asarray(conv1_b, np.float32).reshape(2, 128).T.copy()
    w2 = np.asarray(conv2_w, np.float32).reshape(256, 256, 81)
    w2 = np.ascontiguousarray(w2.transpose(2, 1, 0)).reshape(162, 128, 256).astype(bf)
    b2v = np.asarray(conv2_b, np.float32).reshape(2, 128).T.copy()
    Wf = np.asarray(W, np.float32)
    # wlb rows q = f%128, cols t*160 + o*10 + c  (co-order is (o, c))
    wl = np.ascontiguousarray(Wf.transpose(0, 3, 2, 1)).reshape(KT, 128, CO)
    wl = np.ascontiguousarray(wl.transpose(1, 0, 2)).reshape(128, KT * CO).astype(bf)
    s8x = np.zeros((128, 16), np.float32)
    s8x[np.arange(128), np.arange(128) // 8] = 1.0
    e2 = np.zeros((64, 128), np.float32)
    e2[np.arange(128) % 64, np.arange(128)] = 1.0

    shared = {
        "w1t": w1, "b1": b1v, "w2s": w2, "b2": b2v, "wlb": wl,
        "sel8x": s8x.astype(bf), "selc": s8x.T.copy().astype(bf),
        "ones16": np.ones((16, 1), np.float32),
        "ones1": np.ones((1, 16), np.float32),
        "eyeb2": e2.astype(bf),
    }
    in_maps = []
    for c in range(NCORES):
        xs = np.zeros(BL * 784 + 8, bf)
        xs[:BL * 784] = xf[c * BL:(c + 1) * BL].reshape(-1).astype(bf)
        in_maps.append({"xin": xs, **shared})
    return in_maps


def kernel(x, conv1_w, conv1_b, conv2_w, conv2_b, W):
    if "nc" not in _CACHE:
        _CACHE["nc"] = build_nc()
    nc = _CACHE["nc"]
    in_maps = make_in_maps(x, conv1_w, conv1_b, conv2_w, conv2_b, W)
    res = run_bass_kernel_spmd(nc, in_maps, list(range(NCORES)), trace=False)
    outs = [res.results[c]["out"] for c in range(NCORES)]
    full = np.concatenate(outs, axis=0).reshape(B, O, C)
    return np.ascontiguousarray(full.transpose(0, 2, 1)).astype(np.float32)


# revision 32
# speedup vs baseline: 2.3251x; 1.0909x over previous
"""CapsuleNet forward kernel for 8 Trainium2 NeuronCores.

Data-parallel over batch (64 images / core); the routing b_ij batch-mean
uses an AllReduce per iteration.  u_hat is never materialized: s_j and the
agreement mean are computed directly against W from the 9216-dim flattened
capsule vector u.

Per-core pipeline:
  conv1  : K=81 matmuls with garbage-cropped 200-col rhs (im2col built by
           strided DMA from DRAM, 2240B segments); 8 matmul slots per
           4-bank PSUM tile; one fused ReLU+bias copy per 4 images,
           alternating DVE/Act engines
  conv2  : 324 accumulating K=128 matmuls (81 taps x 2 ci chunks) per co
           chunk over the full local batch (5 image-aligned PSUM banks);
           bias-add drains to bf16 and scatter-writes the capsule layout
           to DRAM (72B segments)
  capsule: u2T[p = b + 64*mc, co*36+s] bf16 [128, 4608]; squash over
           8-elem groups (f32 norms, bf16 scale); u2R = xbar DMA-transpose
           (f-major, cols (h, j, b))
  routing: s_j^T = (c-scaled W)^T @ u2, 72 K-tile accumulation;
           agreement mean m = sum_{o,i} W .* (v2^T @ u2) via rank-64
           matmul + DVE mult/group-reduce + selector matmuls;
           AllReduce(m) -> b_ij update -> softmax.
"""

import numpy as np
import ml_dtypes

import concourse.bacc as bacc
import concourse.bass as bass
import concourse.mybir as mybir
import concourse.tile as tile
from concourse.bass_utils import run_bass_kernel_spmd

F32 = mybir.dt.float32
BF16 = mybir.dt.bfloat16
MUL = mybir.AluOpType.mult
ADD = mybir.AluOpType.add
MAX = mybir.AluOpType.max
AXX = mybir.AxisListType.X
ACT = mybir.ActivationFunctionType

NCORES = 8
B = 512
BL = B // NCORES        # 64 images per core
SB = 16                 # conv1 im2col sub-batch
NSB = BL // SB
J = 560                 # 20 rows x 28 cols (8 garbage cols/row)
JC = 400                # compact 20x20 conv1 output per image
R, C, O, I = 1152, 10, 16, 8
F = R * I               # 9216
FH = F // 2             # 4608 per fold half
CO = C * O              # 160
KT = F // 128           # 72
S2 = 36                 # 6x6 conv2 positions per image
N2 = BL * S2
BCH = [(0, 14), (14, 14), (28, 14), (42, 14), (56, 8)]
NIT = 3


def _sub(ap, off, dims):
    """Arbitrary strided view (offset in elements, dims=[[step,count],..])."""
    return bass.AP(ap.tensor, ap.offset + off, [list(d) for d in dims])


def _pp(ap):
    """Partition pitch (elements per partition row) of an SBUF AP."""
    return ap.ap[0][0]


def build_nc(for_sim=False, reps=1):
    nc = bacc.Bacc("TRN2", target_bir_lowering=False, debug=False,
                   num_devices=1 if for_sim else NCORES)
    nc._for_sim = for_sim

    xin = nc.dram_tensor("xin", [BL * 784 + 8], BF16, kind="ExternalInput").ap()
    cpb = nc.dram_tensor("cpb", [128, 528], BF16, kind="ExternalInput").ap()
    cpf = nc.dram_tensor("cpf", [128, 21], F32, kind="ExternalInput").ap()
    w2s = nc.dram_tensor("w2s", [162, 128, 256], BF16, kind="ExternalInput").ap()
    wlb = nc.dram_tensor("wlb", [128, KT * CO], BF16, kind="ExternalInput").ap()
    out = nc.dram_tensor("out", [BL, CO], F32, kind="ExternalOutput").ap()

    cc_in = nc.dram_tensor("cc_in", [16, KT * C], F32)
    cc_out = nc.dram_tensor("cc_out", [16, KT * C], F32,
                            addr_space="Local" if for_sim else "Shared")
    vd = nc.dram_tensor("vd", [128, FH], BF16)         # conv2 out bounce

    with tile.TileContext(nc, num_cores=NCORES) as tc:
        for _rep in range(reps):
            _body(tc, nc, xin, cpb, cpf, w2s, wlb, out, cc_in, cc_out, vd)
    nc.compile()
    return nc


def _body(tc, nc, xin, cpb, cpf, w2s, wlb, out, cc_in, cc_out, vd):
    with tc.tile_pool(name="const", bufs=1) as pc, \
         tc.tile_pool(name="upers", bufs=1) as pU:

        cpb_sb = pc.tile([128, 528], BF16, tag="cpb")
        nc.gpsimd.dma_start(cpb_sb[:], cpb)
        cpf_sb = pc.tile([128, 21], F32, tag="cpf")
        nc.gpsimd.dma_start(cpf_sb[:], cpf)
        w1t_sb = cpb_sb[0:81, 0:256]
        sel8x_sb = cpb_sb[:, 256:272]
        selc_sb = cpb_sb[0:16, 272:400]
        eyeb2_sb = cpb_sb[0:64, 400:528]
        b1_sb = cpf_sb[:, 0:2]
        b2_sb = cpf_sb[:, 2:4]
        ones16_sb = cpf_sb[0:16, 4:5]
        ones1_sb = cpf_sb[0:1, 5:21]
        u2T = pU.tile([128, FH], BF16, tag="u2T")       # folded capsules
        u2R = pU.tile([128, FH], BF16, tag="u2R")       # f-major (h, j, b)
        wsb = pU.tile([128, KT * CO], BF16, tag="wsb")

        # ============ Phase A: conv1 + conv2 + capsule formation ===========
        with tc.tile_pool(name="pH", bufs=1) as pH:
            h1 = [pH.tile([128, BL * JC], BF16, tag=f"h1_{kc}",
                          name=f"h1_{kc}") for kc in range(2)]

            # conv1: 8 matmul slots (4 images x 2 half-rows) per 4-bank
            # PSUM tile; one fused bias+ReLU copy per tile
            with tc.tile_pool(name="pA", bufs=2) as pA, \
                 tc.tile_pool(name="ps1", bufs=4, space="PSUM") as ps1:
                flip = 0
                for half in range(NSB):
                    b0h = half * SB
                    A = pA.tile([81, SB * J], BF16, tag="A")
                    pa = _pp(A[:])
                    # im2col: one DMA per kernel row
                    for kh in range(9):
                        nc.sync.dma_start(
                            _sub(A[:], 9 * kh * pa, [[pa, 9], [J, SB], [1, J]]),
                            _sub(xin, b0h * 784 + 28 * kh,
                                 [[1, 9], [784, SB], [1, J]]))

                    for bg in range(SB // 2):
                        for mc in range(2):
                            lhsT = w1t_sb[:, mc * 128:(mc + 1) * 128]
                            ps = ps1.tile([128, 1024], F32, tag="c1ps")
                            for sl in range(4):      # slot = (img, hf)
                                bi = bg * 2 + sl // 2
                                hf = sl % 2
                                rhs = _sub(A[:], bi * J + hf * 280,
                                           [[pa, 81], [28, 10], [1, 20]])
                                nc.tensor.matmul(
                                    ps[:, sl * 256: sl * 256 + 200],
                                    lhsT, rhs, start=True, stop=True)
                            doff = (b0h + bg * 2) * JC
                            dstc = _sub(h1[mc][:], doff,
                                        [[_pp(h1[mc][:]), 128], [200, 4],
                                         [1, 200]])
                            srcc = _sub(ps[:], 0,
                                        [[_pp(ps[:]), 128], [256, 4], [1, 200]])
                            bb = b1_sb[:, mc:mc + 1]
                            if flip % 2 == 0:
                                nc.vector.tensor_scalar(dstc, srcc, bb, 0.0,
                                                        op0=ADD, op1=MAX)
                            else:
                                nc.scalar.activation(dstc, srcc, ACT.Relu,
                                                     bias=bb)
                            flip += 1

            # routing weights: contiguous loads on the Act DMA queue,
            # floored past conv1's im2col so they don't steal DMA bandwidth
            with tc.tile_wait_until(0.005):
                nc.gpsimd.dma_start(wsb[:], wlb)

            # conv2 (the mc=0 half's squash hides under mc=1's matmuls)
            with tc.tile_pool(name="pW2", bufs=8) as pW2, \
                 tc.tile_pool(name="pV", bufs=2) as pV, \
                 tc.tile_pool(name="squ", bufs=2) as pq0, \
                 tc.tile_pool(name="ps2", bufs=1, space="PSUM") as ps2:
                NGC = 1152 // I          # squash chunk: 1152 f, 144 groups

                def squash_half(h):
                    # squash u2T rows [64h, 64h+64) in 4 column chunks,
                    # then DMA-transpose the half into u2R
                    rows = slice(h * 64, h * 64 + 64)
                    for qc in range(4):
                        fsl = slice(qc * 1152, (qc + 1) * 1152)
                        uv = u2T[rows, fsl]
                        sqr = pq0.tile([128, 1152], F32, tag="sqr", name="sqr")[rows, :]
                        nc.vector.tensor_mul(sqr, uv, uv)
                        sq = pq0.tile([128, NGC], F32, tag="sq", name="sq")[rows, :]
                        nc.vector.tensor_reduce(
                            sq, sqr.rearrange("p (r i) -> p r i", i=I),
                            axis=AXX, op=ADD)
                        srt = pq0.tile([128, NGC], F32, tag="srt", name="srt")[rows, :]
                        nc.scalar.sqrt(srt, sq)
                        d2 = pq0.tile([128, NGC], F32, tag="d2", name="d2")[rows, :]
                        nc.vector.scalar_tensor_tensor(d2, sq, 1.0, srt,
                                                       op0=ADD, op1=MUL)
                        rc = pq0.tile([128, NGC], F32, tag="rc", name="rc")[rows, :]
                        nc.vector.reciprocal(rc, d2)
                        g = pq0.tile([128, NGC], F32, tag="g", name="g")[rows, :]
                        nc.vector.tensor_mul(g, sq, rc)
                        gx = pq0.tile([128, 1152], BF16, tag="gx", name="gx")[rows, :]
                        ppg = _pp(g)
                        ppx = _pp(gx)
                        nc.scalar.activation(
                            _sub(gx, 0, [[ppx, 64], [I, NGC], [1, I]]),
                            _sub(g, 0, [[ppg, 64], [1, NGC], [0, I]]),
                            ACT.Copy)
                        nc.vector.tensor_mul(uv, uv, gx)
                    # u2R[q, h*2304 + j*64 + b] = u2T[b + 64h, j*128 + q]
                    nc.sync.dma_start_transpose(
                        _sub(u2R[:], h * 2304,
                             [[_pp(u2R[:]), 128], [64, 36], [1, 64]]),
                        u2T[rows, :])

                for mc in range(2):
                    pss = [ps2.tile([128, nb * S2], F32, tag=f"c2ps{i}",
                                    name=f"c2ps{i}_{mc}")
                           for i, (_, nb) in enumerate(BCH)]
                    for khw in range(81):
                        kh2, kw2 = khw // 9, khw % 9
                        wch = pW2.tile([128, 512], BF16, tag="wch")
                        nc.sync.dma_start(
                            _sub(wch[:], 0,
                                 [[_pp(wch[:]), 128], [256, 2], [1, 256]]),
                            _sub(w2s, khw * 2 * 128 * 256,
                                 [[256, 128], [128 * 256, 2], [1, 256]]))
                        for kc in range(2):
                            lhsT = wch[:, kc * 256 + mc * 128:
                                        kc * 256 + mc * 128 + 128]
                            for ic, (b0, nb) in enumerate(BCH):
                                rhs = _sub(h1[kc][:], b0 * JC + 20 * kh2 + kw2,
                                           [[_pp(h1[kc][:]), 128],
                                            [JC, nb], [40, 6], [2, 6]])
                                nc.tensor.matmul(
                                    pss[ic][:], lhsT, rhs,
                                    start=(kc == 0 and khw == 0),
                                    stop=(kc == 1 and khw == 80))
                    v = pV.tile([128, N2], BF16, tag="v")
                    for ic, (b0, nb) in enumerate(BCH):
                        nc.scalar.activation(v[:, b0 * S2:(b0 + nb) * S2],
                                             pss[ic][:], ACT.Identity,
                                             bias=b2_sb[:, mc:mc + 1])
                    # scatter-write the capsule layout: vd[b+64mc, co*36+s]
                    vdst = _sub(vd.ap(), mc * 64 * FH,
                                [[36, 128], [FH, 64], [1, 36]])
                    nc.sync.dma_start(vdst, v[:])
                    # contiguous read back into the folded SBUF tile
                    nc.sync.dma_start(u2T[mc * 64:(mc + 1) * 64, :],
                                      _sub(vd.ap(), mc * 64 * FH,
                                           [[FH, 64], [1, FH]]))
                    squash_half(mc)

        if True:
            # ============ routing ==========================================
            # co-order is (o, c): co' = o*10 + c. m/b_ij/csm live in the
            # [16 r_local, 72 t * 10 c] layout; r = 16t + r_local.
            with tc.tile_pool(name="pB", bufs=1) as pB, \
                 tc.tile_pool(name="pTb", bufs=3) as pTb, \
                 tc.tile_pool(name="pPm", bufs=3) as pPm, \
                 tc.tile_pool(name="psq2", bufs=1) as pq, \
                 tc.tile_pool(name="psB", bufs=2, space="PSUM") as psB, \
                 tc.tile_pool(name="psS", bufs=1, space="PSUM") as psS:

                wp = pB.tile([128, KT * CO], BF16, tag="wp")
                adum = pB.tile([1, 2], F32, tag="adum")
                cE = pB.tile([128, KT * C], BF16, tag="cE")
                bijA = pB.tile([16, KT * C], F32, tag="bijA")
                bijB = pB.tile([16, KT * C], F32, tag="bijB")
                exp16 = pB.tile([16, KT * C], F32, tag="exp16")
                csm16 = pB.tile([16, KT * C], BF16, tag="csm16")
                mAllN = pB.tile([16, KT * C], F32, tag="mAllN")
                msum = pB.tile([16, KT * C], F32, tag="msum")
                pro = pB.tile([128, KT * C], BF16, tag="pro")
                v2T = pB.tile([BL, CO], F32, tag="v2T")
                v2Tb = pB.tile([BL, CO], BF16, tag="v2Tb")
                v2rep = pB.tile([128, CO], BF16, tag="v2rep")

                lam = 1.0 / R
                for it in range(NIT):
                    if it > 0:
                        # cE[8*rl+i, (t,c)] = csm16[rl, (t,c)] via selector
                        ceps = psB.tile([128, 1024], F32, tag="ceps", bufs=1)
                        for hf2 in range(2):
                            nc.tensor.matmul(
                                ceps[:, hf2 * 512: hf2 * 512 + 360],
                                selc_sb, csm16[:, hf2 * 360:(hf2 + 1) * 360],
                                start=True, stop=True)
                            nc.scalar.activation(
                                cE[:, hf2 * 360:(hf2 + 1) * 360],
                                ceps[:, hf2 * 512: hf2 * 512 + 360], ACT.Copy)
                        # wp = wsb * broadcast_o(cE), two 2x half-passes so
                        # s_j's first K-tiles can start after half A
                        ppw = _pp(wp[:])
                        pps = _pp(wsb[:])
                        ppe = _pp(cE[:])
                        for wh in range(2):
                            nc.vector.tensor_tensor(
                                _sub(wp[:], wh * 36 * CO,
                                     [[ppw, 128], [CO, 36], [C, O], [1, C]]),
                                _sub(wsb[:], wh * 36 * CO,
                                     [[pps, 128], [CO, 36], [C, O], [1, C]]),
                                _sub(cE[:], wh * 36 * C,
                                     [[ppe, 128], [C, 36], [0, O], [1, C]]),
                                op=MUL)

                    # s_j^T [b, (o,c)] over 72 accumulating K-tiles
                    wcur = wsb if it == 0 else wp
                    ssum = psS.tile([128, CO], F32, tag="sv")
                    for t in range(KT):
                        j, h = t % 36, t // 36
                        lhsT = _sub(u2R[:], h * 2304 + j * 64,
                                    [[_pp(u2R[:]), 128], [1, BL]])
                        nc.tensor.matmul(ssum[0:BL, :], lhsT,
                                         wcur[:, t * CO:(t + 1) * CO],
                                         start=(t == 0), stop=(t == KT - 1))

                    # v2 = squash(s) over o-groups (iter0 folds 1/R)
                    ssb = pq.tile([BL, CO], F32, tag="ssb")
                    nc.vector.tensor_copy(ssb[:], ssum[0:BL, :])
                    svr = pq.tile([BL, CO], F32, tag="svr")
                    nc.vector.tensor_mul(svr[:], ssb[:], ssb[:])
                    sqv = pq.tile([BL, C], F32, tag="sqv")
                    ppsv = _pp(svr[:])
                    nc.vector.tensor_reduce(
                        sqv[:],
                        _sub(svr[:], 0, [[ppsv, BL], [1, C], [C, O]]),
                        axis=AXX, op=ADD)
                    if it == 0:
                        nc.vector.tensor_scalar(sqv[:], sqv[:], lam * lam,
                                                None, op0=MUL)
                    srtv = pq.tile([BL, C], F32, tag="srtv")
                    nc.scalar.sqrt(srtv[:], sqv[:])
                    dv2 = pq.tile([BL, C], F32, tag="dv2")
                    nc.vector.scalar_tensor_tensor(dv2[:], sqv[:], 1.0,
                                                   srtv[:], op0=ADD, op1=MUL)
                    rcv = pq.tile([BL, C], F32, tag="rcv")
                    nc.vector.reciprocal(rcv[:], dv2[:])
                    gv = pq.tile([BL, C], F32, tag="gv")
                    nc.vector.tensor_mul(gv[:], sqv[:], rcv[:])
                    if it == 0:
                        nc.vector.tensor_scalar(gv[:], gv[:], lam, None,
                                                op0=MUL)
                    ppv = _pp(v2T[:])
                    pps2 = _pp(ssb[:])
                    ppgv = _pp(gv[:])
                    nc.vector.tensor_tensor(
                        _sub(v2T[:], 0, [[ppv, BL], [C, O], [1, C]]),
                        _sub(ssb[:], 0, [[pps2, BL], [C, O], [1, C]]),
                        _sub(gv[:], 0, [[ppgv, BL], [0, O], [1, C]]),
                        op=MUL)

                    if it == NIT - 1:
                        # out stays in (o, c) order; host transposes
                        nc.sync.dma_start(out, v2T[:])
                        break

                    # v replicated to both partition halves via PE selector
                    nc.scalar.activation(v2Tb[:], v2T[:], ACT.Copy)
                    vrp = psS.tile([128, CO], F32, tag="sv")
                    nc.tensor.matmul(vrp[:], eyeb2_sb, v2Tb[:],
                                     start=True, stop=True)
                    nc.scalar.activation(v2rep[:], vrp[:], ACT.Copy)

                    # agreement: T'f[f, (o,c)] = sum_b u[b,f] v[b,(o,c)],
                    # then m16[rl, (t,c)] = sum_{i,o} wsb .* T'f
                    for ggr in range(KT // 3):
                        tfp = psB.tile([128, 480], F32, tag="tfp")
                        for dt3 in range(3):
                            t = ggr * 3 + dt3
                            j, h = t % 36, t // 36
                            lhsT = u2T[h * 64:(h + 1) * 64,
                                       j * 128:(j + 1) * 128]
                            nc.tensor.matmul(tfp[:, dt3 * CO:(dt3 + 1) * CO],
                                             lhsT,
                                             v2rep[h * 64:(h + 1) * 64, :],
                                             start=True, stop=True)
                        tpb = pTb.tile([128, 480], BF16, tag="tpb")
                        nc.scalar.activation(tpb[:], tfp[:], ACT.Copy)
                        pm = pPm.tile([128, 480], BF16, tag="pm")
                        mule = nc.vector if ggr % 2 == 0 else nc.gpsimd
                        mule.tensor_tensor(
                            pm[:], tpb[:], wsb[:, ggr * 480:(ggr + 1) * 480],
                            op=MUL)
                        ppm = _pp(pm[:])
                        ppr = _pp(pro[:])
                        with nc.allow_low_precision(reason="m16 in bf16 ok"):
                            nc.vector.tensor_reduce(
                                _sub(pro[:], ggr * 30,
                                     [[ppr, 128], [10, 3], [1, 10]]),
                                _sub(pm[:], 0,
                                     [[ppm, 128], [CO, 3], [1, C], [C, O]]),
                                axis=AXX, op=ADD)
                    # i-sum via selector: m16[rl, (t,c)]
                    m16p = psB.tile([16, 1024], F32, tag="m16p", bufs=1)
                    for hf2 in range(2):
                        nc.tensor.matmul(
                            m16p[:, hf2 * 512: hf2 * 512 + 360],
                            sel8x_sb, pro[:, hf2 * 360:(hf2 + 1) * 360],
                            start=True, stop=True)
                    ppmp = _pp(m16p[:])
                    nc.vector.tensor_copy(
                        mAllN[:],
                        _sub(m16p[:], 0, [[ppmp, 16], [512, 2], [1, 360]]))

                    nc.scalar.activation(adum[:, 0:1], adum[:, 1:2], ACT.Exp)
                    nc.sync.dma_start(cc_in.ap(), mAllN[:])
                    if getattr(nc, "_for_sim", False):
                        nc.sync.dma_start(cc_out.ap(), cc_in.ap())
                    else:
                        nc.gpsimd.collective_compute(
                            "AllReduce", ADD,
                            replica_groups=[list(range(NCORES))],
                            ins=[cc_in.ap()], outs=[cc_out.ap()])
                    nc.sync.dma_start(msum[:], cc_out.ap())
                    bij = bijA if it == 0 else bijB
                    if it == 0:
                        nc.vector.tensor_scalar(bij[:], msum[:], 1.0 / B,
                                                None, op0=MUL)
                    else:
                        nc.vector.scalar_tensor_tensor(
                            bij[:], msum[:], 1.0 / B, bijA[:],
                            op0=MUL, op1=ADD)
                    # softmax over routes r = (rl, t); no max-subtraction
                    # (|b_ij| stays O(1), far from the exp overflow range)
                    nc.scalar.activation(exp16[:], bij[:], ACT.Exp)
                    tsum = pq.tile([16, C], F32, tag="tsum")
                    ppb = _pp(exp16[:])
                    nc.vector.tensor_reduce(
                        tsum[:],
                        _sub(exp16[:], 0, [[ppb, 16], [1, C], [C, KT]]),
                        axis=AXX, op=ADD)
                    # partition-sum (16 -> 1) and broadcast back (1 -> 16)
                    dsp = psS.tile([16, 128], F32, tag="dsp")
                    nc.tensor.matmul(dsp[0:1, 0:C], ones16_sb, tsum[:],
                                     start=True, stop=True)
                    rcp = pq.tile([1, C], F32, tag="rcp")
                    nc.vector.reciprocal(rcp[:], dsp[0:1, 0:C])
                    nc.tensor.matmul(dsp[0:16, 64:64 + C], ones1_sb,
                                     rcp[:], start=True, stop=True)
                    ppd = _pp(dsp[:])
                    nc.vector.tensor_tensor(
                        csm16[:], exp16[:],
                        _sub(dsp[:], 64, [[ppd, 16], [0, KT], [1, C]]),
                        op=MUL)
                    nc.scalar.sqrt(adum[:, 0:1], adum[:, 1:2])


# ------------------------- host side ---------------------------------------
_CACHE = {}


def make_in_maps(x, conv1_w, conv1_b, conv2_w, conv2_b, W):
    bf = ml_dtypes.bfloat16
    xf = np.ascontiguousarray(np.asarray(x, np.float32).reshape(B, 784))
    w1 = np.ascontiguousarray(
        np.asarray(conv1_w, np.float32).reshape(256, 81).T).astype(bf)
    b1v = np.asarray(conv1_b, np.float32).reshape(2, 128).T.copy()
    w2 = np.asarray(conv2_w, np.float32).reshape(256, 256, 81)
    w2 = np.ascontiguousarray(w2.transpose(2, 1, 0)).reshape(162, 128, 256).astype(bf)
    b2v = np.asarray(conv2_b, np.float32).reshape(2, 128).T.copy()
    cpb = np.zeros((128, 528), bf)
    cpf = np.zeros((128, 21), np.float32)
    cpb[0:81, 0:256] = w1
    cpf[:, 0:2] = b1v
    cpf[:, 2:4] = b2v
    Wf = np.asarray(W, np.float32)
    # wlb rows q = f%128, cols t*160 + o*10 + c  (co-order is (o, c))
    wl = np.ascontiguousarray(Wf.transpose(0, 3, 2, 1)).reshape(KT, 128, CO)
    wl = np.ascontiguousarray(wl.transpose(1, 0, 2)).reshape(128, KT * CO).astype(bf)
    s8x = np.zeros((128, 16), np.float32)
    s8x[np.arange(128), np.arange(128) // 8] = 1.0
    e2 = np.zeros((64, 128), np.float32)
    e2[np.arange(128) % 64, np.arange(128)] = 1.0
    cpb[:, 256:272] = s8x.astype(bf)
    cpb[0:16, 272:400] = s8x.T.astype(bf)
    cpb[0:64, 400:528] = e2.astype(bf)
    cpf[0:16, 4:5] = 1.0
    cpf[0:1, 5:21] = 1.0

    shared = {"cpb": cpb, "cpf": cpf, "w2s": w2, "wlb": wl}
    in_maps = []
    for c in range(NCORES):
        xs = np.zeros(BL * 784 + 8, bf)
        xs[:BL * 784] = xf[c * BL:(c + 1) * BL].reshape(-1).astype(bf)
        in_maps.append({"xin": xs, **shared})
    return in_maps


def kernel(x, conv1_w, conv1_b, conv2_w, conv2_b, W):
    if "nc" not in _CACHE:
        _CACHE["nc"] = build_nc()
    nc = _CACHE["nc"]
    in_maps = make_in_maps(x, conv1_w, conv1_b, conv2_w, conv2_b, W)
    res = run_bass_kernel_spmd(nc, in_maps, list(range(NCORES)), trace=False)
    outs = [res.results[c]["out"] for c in range(NCORES)]
    full = np.concatenate(outs, axis=0).reshape(B, O, C)
    return np.ascontiguousarray(full.transpose(0, 2, 1)).astype(np.float32)
